# revision 2
# baseline (speedup 1.0000x reference)
"""Trainium2 Bass kernel for: out = A @ dequant_int4(weight, weight_scale) + bias.

Problem shapes (fp32 A, packed-int4 weight):
    A            [8192, 4096] f32
    weight       [2048, 11008] u8   (two int4 nibbles per byte along K;
                                     row 2i = low nibble, row 2i+1 = high nibble)
    weight_scale [128, 11008] f32   (per-group scale, group_size=32 along K)
    bias         [11008] f32
    out          [8192, 11008] f32

Sharding: tensor-parallel along out_features N across 8 NeuronCores.
Each core gets the full A, a 1376-wide column slice of weight/scale/bias and
computes its [8192, 1376] output slice; the host concatenates slices.

Host prep (layout only): the packed int4 bytes are unpacked to one nibble per
u8 byte in natural k-order [4096, N], scales are repeated 32x to per-k rows
(bf16), bias replicated to [128, ns].  All arithmetic (bias-8 shift, scale
multiply, GEMM, bias add) happens on device.

Per-core kernel strategy:
  - Dequant once into a resident SBUF buffer wsb[p, kb, n] (natural k-order:
    k = 128*kb + p) with two DVE ops per element: (nib - 8) -> bf16, * scale.
  - Per 128-row chunk of A: SWDGE cast-DMA (f32->bf16) the natural [128, 4096]
    tile, then an xbar DMA-transpose (SBUF->SBUF, ~3.6us, on DMA queues) into
    at[p, kb, m] = A[m0+m, 128*kb+p].  The PE runs pure matmuls: 3 n-chunks
    x 32 k-blocks accumulating in PSUM.  Bias is added during the PSUM->SBUF
    eviction on the DVE.
"""

import numpy as np
import ml_dtypes

import concourse.bacc as bacc
import concourse.tile as tile
from concourse import mybir
from concourse.bass_utils import run_bass_kernel_spmd

M, K, N = 8192, 4096, 11008
NCORES = 8
NS = N // NCORES  # 1376 out-features per core
P = 128
NKB = K // P      # 32 k-blocks
SB = 4            # k-blocks per dequant super-DMA
N_CHUNKS = [(0, 512), (512, 512), (1024, 352)]


def build_nc(m=M, ns=NS, debug=False):
    """Build the per-core Bass program (identical on all cores)."""
    mch = m // P

    nc = bacc.Bacc(None, target_bir_lowering=False, debug=debug)
    A = nc.dram_tensor("A", [m, K], mybir.dt.float32, kind="ExternalInput")
    NIB = nc.dram_tensor("nib", [K, ns], mybir.dt.uint8, kind="ExternalInput")
    SREP = nc.dram_tensor("srep", [K, ns], mybir.dt.bfloat16, kind="ExternalInput")
    BIAS = nc.dram_tensor("bias", [P, ns], mybir.dt.float32, kind="ExternalInput")
    OUT = nc.dram_tensor("out", [m, ns], mybir.dt.float32, kind="ExternalOutput")

    with tile.TileContext(nc) as tc:
        with (
            tc.tile_pool(name="singles", bufs=1) as singles,
            tc.tile_pool(name="wpool", bufs=1) as wpool,
            tc.tile_pool(name="dq", bufs=2) as dq,
            tc.tile_pool(name="apool", bufs=3) as apool,
            tc.tile_pool(name="atpool", bufs=3) as atpool,
            tc.tile_pool(name="opool", bufs=3) as opool,
            tc.tile_pool(name="psum_o", bufs=4, space="PSUM") as psum_o,
        ):
            bias_t = singles.tile([P, ns], mybir.dt.float32)
            nc.sync.dma_start(out=bias_t, in_=BIAS[:, :])

            # ---- one-shot dequant into resident SBUF, natural k-order ----
            wsb = wpool.tile([P, NKB, ns], mybir.dt.bfloat16)
            for sb in range(NKB // SB):
                pk = dq.tile([P, SB, ns], mybir.dt.uint8, tag="pk")
                st = dq.tile([P, SB, ns], mybir.dt.bfloat16, tag="st")
                rows = slice(sb * SB * P, (sb + 1) * SB * P)
                nc.sync.dma_start(
                    out=pk, in_=NIB[rows, :].rearrange("(b p) n -> p b n", p=P))
                nc.sync.dma_start(
                    out=st, in_=SREP[rows, :].rearrange("(b p) n -> p b n", p=P))
                for j in range(SB):
                    kb = sb * SB + j
                    v = dq.tile([P, ns], mybir.dt.bfloat16, tag="v")
                    nc.vector.tensor_scalar(
                        out=v, in0=pk[:, j, :], scalar1=8, scalar2=None,
                        op0=mybir.AluOpType.subtract)
                    nc.vector.tensor_tensor(
                        out=wsb[:, kb, :], in0=v, in1=st[:, j, :],
                        op=mybir.AluOpType.mult)

            # ---- main loop over 128-row chunks of A ----
            for mc in range(mch):
                a_nat = apool.tile([P, K], mybir.dt.bfloat16)
                nc.gpsimd.dma_start(out=a_nat, in_=A[mc * P:(mc + 1) * P, :])  # f32->bf16
                at = atpool.tile([P, NKB, P], mybir.dt.bfloat16)
                nc.sync.dma_start_transpose(at[:, :, :], a_nat[:, :])

                o_sb = opool.tile([P, ns], mybir.dt.float32)
                for (n0, nch) in N_CHUNKS:
                    po = psum_o.tile([P, 512], mybir.dt.float32, tag="po")
                    for kb in range(NKB):
                        nc.tensor.matmul(
                            po[:, :nch], lhsT=at[:, kb, :], rhs=wsb[:, kb, n0:n0 + nch],
                            start=(kb == 0), stop=(kb == NKB - 1))
                    nc.vector.tensor_tensor(
                        out=o_sb[:, n0:n0 + nch], in0=po[:, :nch],
                        in1=bias_t[:, n0:n0 + nch], op=mybir.AluOpType.add)
                nc.sync.dma_start(out=OUT[mc * P:(mc + 1) * P, :], in_=o_sb)

    nc.finalize()
    return nc


_NC_CACHE = {}


def _get_nc():
    if "nc" not in _NC_CACHE:
        _NC_CACHE["nc"] = build_nc()
    return _NC_CACHE["nc"]


def shard_inputs(A, weight, weight_scale, bias):
    A = np.ascontiguousarray(np.asarray(A, dtype=np.float32))
    wq = np.asarray(weight, dtype=np.uint8)
    ws = np.asarray(weight_scale, dtype=np.float32)
    bs = np.asarray(bias, dtype=np.float32)

    # unpack nibbles to natural k-order: row 2i = low nibble, row 2i+1 = high
    nib = np.empty((K, N), dtype=np.uint8)
    nib[0::2] = wq & 15
    nib[1::2] = wq >> 4
    srep = np.repeat(ws, K // ws.shape[0], axis=0).astype(ml_dtypes.bfloat16)

    in_maps = []
    for c in range(NCORES):
        sl = slice(c * NS, (c + 1) * NS)
        in_maps.append({
            "A": A,
            "nib": np.ascontiguousarray(nib[:, sl]),
            "srep": np.ascontiguousarray(srep[:, sl]),
            # partition-replicated so the device DMA is a plain 2D copy
            "bias": np.ascontiguousarray(np.broadcast_to(bs[sl], (P, NS))),
        })
    return in_maps


def run(inputs, trace=False, **kw):
    nc = _get_nc()
    in_maps = shard_inputs(**inputs)
    res = run_bass_kernel_spmd(nc, in_maps, core_ids=list(range(NCORES)), trace=trace, **kw)
    out = np.concatenate([res.results[c]["out"] for c in range(NCORES)], axis=1)
    return out, res


def kernel(A, weight, weight_scale, bias):
    out, _ = run(dict(A=A, weight=weight, weight_scale=weight_scale, bias=bias))
    return out
